# revision 8
# baseline (speedup 1.0000x reference)
"""Trainium2 Bass kernel for nn_DegModel (EDSR-style degradation backbone +
per-pixel KPN), distributed over 8 NeuronCores.

Sharding: one core per (batch, image-half): core i -> batch i//2, half i%2.
Each core runs the whole backbone locally on its 64-row half plus a 17-row
recomputed halo, so no collectives are needed. Bottom halves are processed
vertically flipped (host flips z and the dy axis of the conv weights, both
per-core input data), which makes the on-device geometry identical for all
cores. The only cross-core quantity — the global mean of the predicted noise
channel — is reduced on host from per-core partial sums.

Feature maps live in SBUF as [128 partitions, J slots, 130] with partition
p = channel + 64*parity and the odd-row half skewed one slot down:
lower[c, j] = F[c, 2j-2], upper[c, j] = F[c, 2j-3]. With this skew a 3x3 conv
over an 8-row output block is exactly 6 full K=128 x M=128 float32r matmuls
(2 per kernel column dx) into one [128, 4, 128] PSUM bank: M columns 0:64
produce the even output rows, 64:128 the odd rows.

Residual blocks ping-pong between two trunk buffers (the z staging buffer is
recycled as the second trunk) so the residual add is out-of-place:
vector (lower half, directly from PSUM) + gpsimd (upper half, via an ACT
copy, since gpsimd cannot read PSUM).

conv_out (1x1) + softmax + the 21x21 KPN run per output row in
pixel-partition layout: feat[:, slot, 1:129] is the stationary operand
(M = 128 pixels) against moving w_out [128, 442], landing logits as
[pixel, channel]; exp lands on the scalar engine with accum_out giving the
softmax denominator for free. The 441-tap weighted sum runs as a bf16
elementwise multiply (DVE 2x mode) followed by a pairwise tree reduction of
tensor_adds (also 2x) down to 28 taps + one final 1x reduce_sum — much
cheaper than a single 1x tensor_reduce over 441. The tree for one of the 3
channels runs on the otherwise-idle gpsimd engine. Softmax normalization is
folded to after the KPN sum (y = sum(patch * exp) / sum(exp)).
"""

import sys

sys.path.insert(0, "/opt/trn_rl_repo")

import numpy as np

import concourse.bass as bass
import concourse.tile as tile
from concourse import mybir
from concourse.bass_utils import run_bass_kernel_spmd

KSIZE = 21
NF = 64
NB = 8
IN_NC = 3
B, H, W = 4, 512, 512
h = w = 128
NCH = KSIZE * KSIZE + 1  # 442
KKP = 448                # padded tap count (zero-padded 441 -> 448)

N_CORES = 8
J = 44    # feature-buffer slots (2 image rows per slot)
X = 130   # 128 cols + 2 zero pad cols

NMID = 2 * NB

_cache = {}


def _enable_ldw_opt():
    import concourse.bass_utils as _bu
    if getattr(_bu, "_ldw_opt_patched", False):
        return
    _orig = _bu.run_command

    def _patched(cmd, **kw):
        if isinstance(cmd, list):
            cmd = ["--enable-ldw-opt=true" if c == "--enable-ldw-opt=false"
                   else c for c in cmd]
        return _orig(cmd, **kw)

    _bu.run_command = _patched
    _bu._ldw_opt_patched = True


def _legalize_waits(nc):
    """This walrus build rejects >1 sync wait per instruction; move extra
    waits onto same-engine NOPs inserted immediately before (engines are
    in-order, so semantics are preserved)."""
    for fn in nc.m.functions:
        for blk in fn.blocks:
            out, changed = [], False
            for inst in blk.instructions:
                si = inst.sync_info
                if si is not None and len(si.on_wait) > 1:
                    waits = list(si.on_wait)
                    for wt in waits[:-1]:
                        nop = mybir.InstNoOp(
                            name=nc.get_next_instruction_name(),
                            ins=[], outs=[], engine=inst.engine)
                        nop.sync_info = mybir.SyncInfo(on_wait=[wt], on_update=[])
                        out.append(nop)
                        changed = True
                    inst.sync_info = mybir.SyncInfo(
                        on_wait=[waits[-1]], on_update=list(si.on_update))
                out.append(inst)
            if changed:
                blk.instructions = out


def _build_nc(bias2_zero, bout_zero):
    f32 = mybir.dt.float32
    f32r = mybir.dt.float32r
    bf16 = mybir.dt.bfloat16
    nc = bass.Bass()

    zg_e = nc.dram_tensor("zg_e", [IN_NC, 41, 128], f32r, kind="ExternalInput")
    zg_o = nc.dram_tensor("zg_o", [IN_NC, 41, 128], f32r, kind="ExternalInput")
    wl1_in = nc.dram_tensor("wl1_in", [128, 3, 128], f32r, kind="ExternalInput")
    wl2_in = nc.dram_tensor("wl2_in", [128, 3, 128], f32r, kind="ExternalInput")
    wl1_mid = nc.dram_tensor("wl1_mid", [NMID, 128, 3, 128], f32r,
                             kind="ExternalInput")
    wl2_mid = nc.dram_tensor("wl2_mid", [NMID, 128, 3, 128], f32r,
                             kind="ExternalInput")
    wout_lo = nc.dram_tensor("wout_lo", [128, NCH], f32r, kind="ExternalInput")
    wout_hi = nc.dram_tensor("wout_hi", [128, NCH], f32r, kind="ExternalInput")
    biases = nc.dram_tensor("biases", [NMID + 1, 128, 1], f32,
                            kind="ExternalInput")
    bout_r = nc.dram_tensor("bout_r", [1, NCH], f32r, kind="ExternalInput")
    ones_r = nc.dram_tensor("ones_r", [1, 128], f32r, kind="ExternalInput")
    # per-(channel,block) expanded KPN patch windows: xw[c, blk, x0, y, tap]
    # -> per partition (x0) one contiguous 8*448*2 = 7168B descriptor.
    xw = nc.dram_tensor("xw", [IN_NC, 8, 128, 8, KKP], bf16,
                        kind="ExternalInput")

    ydev = nc.dram_tensor("ydev", [128, IN_NC, 64], f32, kind="ExternalOutput")
    nsdev = nc.dram_tensor("nsdev", [128, 64], f32, kind="ExternalOutput")

    with tile.TileContext(nc) as tc:
        wpool = tc.alloc_tile_pool(name="w", bufs=1)
        gpool = tc.alloc_tile_pool(name="g", bufs=1)
        wmpool = tc.alloc_tile_pool(name="wmid", bufs=3)
        tpool = tc.alloc_tile_pool(name="rtmp", bufs=2)
        ppool = tc.alloc_tile_pool(name="patch", bufs=3)
        epool = tc.alloc_tile_pool(name="exp", bufs=2)
        trpool = tc.alloc_tile_pool(name="tree", bufs=1)   # DVE-chained tags
        trpool2 = tc.alloc_tile_pool(name="tree2", bufs=2)  # gpsimd tags
        spool = tc.alloc_tile_pool(name="small", bufs=4)
        psum = tc.alloc_tile_pool(name="ps", bufs=6, space="PSUM")
        psum_o = tc.alloc_tile_pool(name="pso", bufs=2, space="PSUM")

        l1_in = wpool.tile([128, 3, 128], f32r)
        l2_in = wpool.tile([128, 3, 128], f32r)
        wo_lo = wpool.tile([128, NCH], f32r)
        wo_hi = wpool.tile([128, NCH], f32r)
        bias_t = wpool.tile([128, NMID + 1], f32)
        bo_t = wpool.tile([1, NCH], f32r)
        ones_t = wpool.tile([1, 128], f32r)
        nc.sync.dma_start(out=l1_in, in_=wl1_in[:])
        nc.sync.dma_start(out=l2_in, in_=wl2_in[:])
        nc.sync.dma_start(out=wo_lo, in_=wout_lo[:])
        nc.sync.dma_start(out=wo_hi, in_=wout_hi[:])
        nc.sync.dma_start(out=bias_t,
                          in_=biases[:].rearrange("l p one -> p (l one)"))
        nc.sync.dma_start(out=bo_t, in_=bout_r[:])
        nc.sync.dma_start(out=ones_t, in_=ones_r[:])

        g_z = gpool.tile([128, J, X], f32r)
        feat = gpool.tile([128, J, X], f32r)
        t1 = gpool.tile([128, J, X], f32r)
        # g_z must be fully zero (unused input-channel partitions are read by
        # the K=128 contraction of the input conv; NaN * 0-weight = NaN).
        nc.vector.memset(g_z[:].bitcast(mybir.dt.float32), 0.0)
        # feat / t1 only need the pad columns (x=0, x=129, read as the dx
        # halo with nonzero weights) and t1's never-written top slots (read
        # one slot past the shrinking write front) cleared; everything else
        # is overwritten before being read.
        for buf in (feat, t1):
            nc.gpsimd.memset(buf[:, :, 0:1].bitcast(mybir.dt.float32), 0.0)
            nc.gpsimd.memset(buf[:, :, 129:130].bitcast(mybir.dt.float32), 0.0)
        nc.gpsimd.memset(t1[:, 41:44, :].bitcast(mybir.dt.float32), 0.0)
        nc.gpsimd.memset(feat[:, 42:44, :].bitcast(mybir.dt.float32), 0.0)

        # z rows (shard-local, 0..80): even row r -> partitions 0:3 slot
        # r//2+1; odd row r -> partitions 64:67 slot (r+1)//2+1.
        # Host packs zg_e = even rows 0..80 (slots 1..41), zg_o = odd rows
        # 1..79 at slots 2..41 (zg_o[0] stays zero -> slot 1 zero = row -1).
        nc.sync.dma_start(out=g_z[0:IN_NC, 1:42, 1:129], in_=zg_e[:])
        nc.sync.dma_start(out=g_z[64:64 + IN_NC, 1:42, 1:129], in_=zg_o[:])

        relu = mybir.ActivationFunctionType.Relu
        ident = mybir.ActivationFunctionType.Identity

        def conv(src, dst, l1, l2, bias_col, func, k_halo, residual,
                 post_group=None):
            # output region: shard-local rows 0 .. 63 + k_halo -> slots 1..hi
            hi = (64 + k_halo) // 2 + 1      # top slot of even output rows
            blocks = [(s, min(4, hi - s + 1)) for s in range(1, hi + 1, 4)]
            # weight-major inside groups of 5 blocks: consecutive matmuls
            # share the stationary operand so walrus ldw-opt dedups the
            # (serialized, non-overlapping) LDWEIGHTS streams.
            for g0 in range(0, len(blocks), 5):
                grp = blocks[g0:g0 + 5]
                tiles = [psum.tile([128, 4, 128], f32, tag="convps",
                                   name=f"cps_{g0}_{i}")
                         for i in range(len(grp))]
                for wi in range(6):
                    dx, phase = wi % 3, wi // 3
                    wt = (l1 if phase == 0 else l2)[:, dx]
                    for (s0, mc), P in zip(grp, tiles):
                        o = s0 + phase
                        nc.tensor.matmul(
                            P[:, 0:mc], wt,
                            src[0:128, o:o + mc, dx:dx + 128],
                            start=(wi == 0), stop=(wi == 5))
                for (s0, mc), P in zip(grp, tiles):
                    if residual is None:
                        nc.scalar.activation(
                            out=dst[0:64, s0:s0 + mc, 1:129],
                            in_=P[0:64, 0:mc],
                            func=func, bias=bias_col[0:64], scale=1.0)
                        nc.scalar.activation(
                            out=dst[64:128, s0 + 1:s0 + 1 + mc, 1:129],
                            in_=P[64:128, 0:mc],
                            func=func, bias=bias_col[64:128], scale=1.0)
                    else:
                        # lower half: DVE adds PSUM + residual -> dst
                        # (out-of-place; dst is the other trunk buffer).
                        # upper half: ACT copy to SBUF (gpsimd cannot read
                        # PSUM), then gpsimd adds.
                        nc.vector.tensor_add(
                            out=dst[0:64, s0:s0 + mc, 1:129],
                            in0=P[0:64, 0:mc],
                            in1=residual[0:64, s0:s0 + mc, 1:129])
                        tmp = tpool.tile([128, 4, 128], f32, tag="rtmp")
                        bias_hi = 0.0 if bias2_zero else bias_col[64:128]
                        nc.scalar.activation(
                            out=tmp[64:128, 0:mc], in_=P[64:128, 0:mc],
                            func=ident, bias=bias_hi, scale=1.0)
                        nc.gpsimd.tensor_add(
                            out=dst[64:128, s0 + 1:s0 + 1 + mc, 1:129],
                            in0=tmp[64:128, 0:mc],
                            in1=residual[64:128, s0 + 1:s0 + 1 + mc, 1:129])
                if post_group is not None:
                    s_last, mc_last = grp[-1]
                    post_group(s_last + mc_last - 1)

        yacc = spool.tile([128, IN_NC, 64], f32, tag="yacc")
        nsacc = spool.tile([128, 64], f32, tag="nsacc")

        KK = KSIZE * KSIZE
        dma_engines = [nc.sync, nc.scalar, nc.gpsimd]

        def kpn_block(bi, final_feat):
            y0 = bi * 8
            ex2 = epool.tile([128, 8, NCH], bf16, tag="ex")
            ssum2 = spool.tile([128, 8], f32, tag="ssum")
            patches = []
            for c in range(IN_NC):
                patch2 = ppool.tile([128, 8, KKP], bf16, tag="patch")
                dma_engines[c].dma_start(out=patch2, in_=xw[c, bi])
                patches.append(patch2)
            for r in range(8):
                yl = y0 + r
                if yl % 2 == 0:
                    slot, wsel = yl // 2 + 1, wo_lo
                else:
                    slot, wsel = (yl + 1) // 2 + 1, wo_hi
                Po = psum_o.tile([128, NCH], f32, tag="pout")
                nc.tensor.matmul(Po, final_feat[:, slot, 1:129], wsel,
                                 start=True, stop=bout_zero)
                if not bout_zero:
                    nc.tensor.matmul(Po, ones_t, bo_t, start=False, stop=True)
                nc.scalar.activation(out=ex2[:, r], in_=Po,
                                     func=mybir.ActivationFunctionType.Exp,
                                     scale=1.0, accum_out=ssum2[:, r:r + 1])
            rcp2 = spool.tile([128, 8], f32, tag="rcp")
            nc.vector.reciprocal(out=rcp2, in_=ssum2)
            ex2v = ex2[:, :, 0:KK]
            for c in range(IN_NC):
                eng = nc.gpsimd if c == 2 else nc.vector
                pool_c = trpool2 if c == 2 else trpool
                prod2 = pool_c.tile([128, 8, KKP], bf16, tag=f"prod{c}")
                # taps 441..447 of patch are zero -> prod padding is zero
                nc.vector.tensor_mul(out=prod2[:, :, 0:KK], in0=ex2v,
                                     in1=patches[c][:, :, 0:KK])
                nc.vector.tensor_mul(out=prod2[:, :, KK:KKP],
                                     in0=patches[c][:, :, KK:KKP],
                                     in1=patches[c][:, :, KK:KKP])
                # pairwise tree at DVE 2x (vs 1x tensor_reduce); scratch is a
                # two-tile pyramid: s1 <- halves of prod, then reuse dead
                # ranges of prod/s1 for the later (smaller) levels.
                s1 = pool_c.tile([128, 8, 224], bf16, tag=f"s1_{c}")
                eng.tensor_add(out=s1, in0=prod2[:, :, 0:224],
                               in1=prod2[:, :, 224:448])
                eng.tensor_add(out=prod2[:, :, 0:112], in0=s1[:, :, 0:112],
                               in1=s1[:, :, 112:224])
                eng.tensor_add(out=s1[:, :, 0:56], in0=prod2[:, :, 0:56],
                               in1=prod2[:, :, 56:112])
                eng.tensor_add(out=prod2[:, :, 0:28], in0=s1[:, :, 0:28],
                               in1=s1[:, :, 28:56])
                pc2 = spool.tile([128, 8, 1], f32, tag=f"pc{c}")
                nc.vector.reduce_sum(out=pc2, in_=prod2[:, :, 0:28],
                                     axis=mybir.AxisListType.X)
                nc.vector.tensor_mul(out=yacc[:, c, y0:y0 + 8],
                                     in0=pc2[:, :, 0], in1=rcp2)
            nc.vector.tensor_mul(out=nsacc[:, y0:y0 + 8],
                                 in0=ex2[:, :, NCH - 1], in1=rcp2)

        conv(g_z, feat, l1_in, l2_in, bias_t[:, 0:1], ident, 16, None)
        trunk = [feat, g_z]   # ping-pong; g_z recycled (pads already zero)
        for rb in range(NB):
            src = trunk[rb % 2]
            dst = trunk[(rb + 1) % 2]
            la, lb = 2 * rb, 2 * rb + 1
            w1a = wmpool.tile([128, 3, 128], f32r, tag="w1")
            w2a = wmpool.tile([128, 3, 128], f32r, tag="w2")
            nc.sync.dma_start(out=w1a, in_=wl1_mid[la])
            nc.sync.dma_start(out=w2a, in_=wl2_mid[la])
            conv(src, t1, w1a, w2a,
                 bias_t[:, 1 + la:2 + la], relu, 15 - 2 * rb, None)
            w1b = wmpool.tile([128, 3, 128], f32r, tag="w1")
            w2b = wmpool.tile([128, 3, 128], f32r, tag="w2")
            nc.sync.dma_start(out=w1b, in_=wl1_mid[lb])
            nc.sync.dma_start(out=w2b, in_=wl2_mid[lb])
            if rb < NB - 1:
                conv(t1, dst, w1b, w2b,
                     bias_t[:, 1 + lb:2 + lb], ident, 14 - 2 * rb, src)
            else:
                # final conv: interleave KPN blocks as their slots complete
                final_feat = dst
                state = {"next": 0}

                def on_group(s_done, _ff=final_feat, _st=state):
                    # kpn block b needs slots up to 4b+5 (odd rows skew +1)
                    while _st["next"] < 8 and 4 * _st["next"] + 5 <= s_done:
                        kpn_block(_st["next"], _ff)
                        _st["next"] += 1

                conv(t1, dst, w1b, w2b,
                     bias_t[:, 1 + lb:2 + lb], ident, 14 - 2 * rb, src,
                     post_group=on_group)
                while state["next"] < 8:
                    kpn_block(state["next"], final_feat)
                    state["next"] += 1

        nc.sync.dma_start(out=ydev[:], in_=yacc)
        nc.sync.dma_start(out=nsdev[:], in_=nsacc)

        for p in (psum_o, psum, spool, trpool2, trpool, epool, ppool, tpool,
                  wmpool, gpool, wpool):
            p.release()

    _legalize_waits(nc)
    return nc


def _stack_l1l2(Wl):
    # Wl [64o, ic, 3, 3] -> L1, L2 [128, 3, 128]
    ic = Wl.shape[1]
    L1 = np.zeros((128, 3, 128), np.float32)
    L2 = np.zeros((128, 3, 128), np.float32)
    for dx in range(3):
        L1[0:ic, dx, 0:64] = Wl[:, :, 1, dx].T
        L1[64:64 + ic, dx, 0:64] = Wl[:, :, 0, dx].T
        L1[0:ic, dx, 64:128] = Wl[:, :, 0, dx].T
        L2[64:64 + ic, dx, 0:64] = Wl[:, :, 2, dx].T
        L2[0:ic, dx, 64:128] = Wl[:, :, 2, dx].T
        L2[64:64 + ic, dx, 64:128] = Wl[:, :, 1, dx].T
    return L1, L2


def _prep_weights(w_in, w1s, w2s, w_out, flip):
    if flip:
        w_in = w_in[:, :, ::-1, :]
        w1s = w1s[:, :, :, ::-1, :]
        w2s = w2s[:, :, :, ::-1, :]
    l1_in, l2_in = _stack_l1l2(w_in)
    L1m = np.zeros((NMID, 128, 3, 128), np.float32)
    L2m = np.zeros((NMID, 128, 3, 128), np.float32)
    for rb in range(NB):
        L1m[2 * rb], L2m[2 * rb] = _stack_l1l2(w1s[rb])
        L1m[2 * rb + 1], L2m[2 * rb + 1] = _stack_l1l2(w2s[rb])
    wo = w_out[:, :, 0, 0]  # [442, 64]
    wlo = np.zeros((128, NCH), np.float32)
    whi = np.zeros((128, NCH), np.float32)
    wlo[0:64] = wo.T
    whi[64:128] = wo.T
    return l1_in, l2_in, L1m, L2m, wlo, whi


def kernel(x, z, eps, w_in, b_in, w1s, b1s, w2s, b2s, w_out, b_out):
    x = np.ascontiguousarray(np.asarray(x, np.float32))
    z = np.asarray(z, np.float32)
    eps = np.asarray(eps, np.float32)
    w_in = np.asarray(w_in, np.float32)
    b_in = np.asarray(b_in, np.float32)
    w1s = np.asarray(w1s, np.float32)
    b1s = np.asarray(b1s, np.float32)
    w2s = np.asarray(w2s, np.float32)
    b2s = np.asarray(b2s, np.float32)
    w_out = np.asarray(w_out, np.float32)
    b_out = np.asarray(b_out, np.float32)

    bias2_zero = bool(np.all(b2s == 0))
    bout_zero = bool(np.all(b_out == 0))
    _enable_ldw_opt()
    key = (bias2_zero, bout_zero)
    if key not in _cache:
        _cache[key] = _build_nc(bias2_zero, bout_zero)
    nc = _cache[key]

    weights = {}
    for flip in (False, True):
        l1_in, l2_in, L1m, L2m, wlo, whi = _prep_weights(
            w_in, w1s, w2s, w_out, flip)
        weights[flip] = (l1_in, l2_in, L1m, L2m, wlo, whi)

    biases = np.zeros((NMID + 1, 128, 1), np.float32)
    biases[0, 0:64, 0] = b_in
    biases[0, 64:128, 0] = b_in
    for rb in range(NB):
        biases[1 + 2 * rb, 0:64, 0] = b1s[rb]
        biases[1 + 2 * rb, 64:128, 0] = b1s[rb]
        biases[2 + 2 * rb, 0:64, 0] = b2s[rb]
        biases[2 + 2 * rb, 64:128, 0] = b2s[rb]
    bout_row = np.ascontiguousarray(b_out.reshape(1, NCH))
    ones_row = np.ones((1, 128), np.float32)

    # padded x (vertical dim only logical; we slice rows directly)
    in_maps = []
    for core in range(N_CORES):
        b, half = core // 2, core % 2
        flip = half == 1
        # shard-local z rows 0..80: top zl[r] = z[b, r]; bottom z flipped
        zl = z[b] if not flip else z[b, :, ::-1]
        zg_e = np.zeros((IN_NC, 41, 128), np.float32)
        zg_o = np.zeros((IN_NC, 41, 128), np.float32)
        zg_e[:, 0:41] = zl[:, 0:81:2]          # rows 0,2,..,80 -> slots 1..41
        zg_o[:, 1:41] = zl[:, 1:80:2]          # rows 1,3,..,79 -> slots 2..41
        # KPN patch windows, fully expanded per output row:
        # xw[c, blk, x0, y, t] = xp[c, 4*y0(8*blk+y) + t//21? ...] laid out as
        # [3, 8, 128, 8, 448]: tap index t < 441 maps to (t//21, t%21) of the
        # 21x21 window; taps 441..447 are zero padding.
        import ml_dtypes
        xp = np.zeros((IN_NC, H + 2 * 10, W + 2 * 10), dtype=ml_dtypes.bfloat16)
        xp[:, 10:10 + H, 10:10 + W] = x[b]
        y0s = np.arange(64) if not flip else (127 - np.arange(64))
        ridx = (4 * y0s)[:, None] + np.arange(KSIZE)[None, :]   # [64, 21]
        cols = 4 * np.arange(128)[:, None] + np.arange(KSIZE)[None, :]
        sub = xp[:, ridx]                 # [3, 64, 21, 532]
        sub = sub[:, :, :, cols]          # [3, 64, 21, 128, 21]
        xw_arr = np.zeros((IN_NC, 64, 128, KKP), dtype=ml_dtypes.bfloat16)
        xw_arr[:, :, :, 0:KSIZE * KSIZE] = np.transpose(
            sub, (0, 1, 3, 2, 4)).reshape(IN_NC, 64, 128, KSIZE * KSIZE)
        xw_arr = np.ascontiguousarray(
            xw_arr.reshape(IN_NC, 8, 8, 128, KKP).transpose(0, 1, 3, 2, 4))
        l1_in, l2_in, L1m, L2m, wlo, whi = weights[flip]
        in_maps.append({
            "zg_e": zg_e, "zg_o": zg_o,
            "wl1_in": l1_in, "wl2_in": l2_in,
            "wl1_mid": L1m, "wl2_mid": L2m,
            "wout_lo": wlo, "wout_hi": whi,
            "biases": biases, "bout_r": bout_row, "ones_r": ones_row,
            "xw": xw_arr,
        })

    trace = bool(globals().get("TRACE", False))
    res = run_bass_kernel_spmd(nc, in_maps, core_ids=list(range(N_CORES)),
                               trace=trace)
    globals()["_last_result"] = res

    out = np.zeros((B, IN_NC, h, w), np.float32)
    for bb in range(B):
        ns_sum = (float(res.results[2 * bb]["nsdev"].sum())
                  + float(res.results[2 * bb + 1]["nsdev"].sum()))
        mean_ns = ns_sum / (h * w)
        for half in range(2):
            ydev = res.results[2 * bb + half]["ydev"]  # [128, 3, 64]
            yt = np.transpose(ydev, (1, 2, 0))         # [3, 64, 128]
            if half == 0:
                out[bb, :, 0:64, :] = yt
            else:
                out[bb, :, 64:128, :] = yt[:, ::-1, :]
        out[bb] += mean_ns * eps[bb]
    return out


# revision 15
# speedup vs baseline: 1.1111x; 1.1111x over previous
"""Trainium2 Bass kernel for nn_DegModel (EDSR-style degradation backbone +
per-pixel KPN), distributed over 8 NeuronCores.

Sharding: one core per (batch, image-half): core i -> batch i//2, half i%2.
Each core runs the whole backbone locally on its 64-row half plus a 17-row
recomputed halo, so no collectives are needed. Bottom halves are processed
vertically flipped (host flips z and the dy axis of the conv weights, both
per-core input data), which makes the on-device geometry identical for all
cores. The only cross-core quantity — the global mean of the predicted noise
channel — is reduced on host from per-core partial sums.

Feature maps live in SBUF as [128 partitions, J slots, 130] with partition
p = channel + 64*parity and the odd-row half skewed one slot down:
lower[c, j] = F[c, 2j-2], upper[c, j] = F[c, 2j-3]. With this skew a 3x3 conv
over an 8-row output block is exactly 6 full K=128 x M=128 float32r matmuls
(2 per kernel column dx) into one [128, 4, 128] PSUM bank: M columns 0:64
produce the even output rows, 64:128 the odd rows.

Residual blocks ping-pong between two trunk buffers (the z staging buffer is
recycled as the second trunk) so the residual add is out-of-place:
vector (lower half, directly from PSUM) + gpsimd (upper half, via an ACT
copy, since gpsimd cannot read PSUM).

conv_out (1x1) + softmax + the 21x21 KPN run per output row in
pixel-partition layout: feat[:, slot, 1:129] is the stationary operand
(M = 128 pixels) against moving w_out [128, 442], landing logits as
[pixel, channel]; exp lands on the scalar engine with accum_out giving the
softmax denominator for free. The 441-tap weighted sum runs as a bf16
elementwise multiply (DVE 2x mode) followed by a pairwise tree reduction of
tensor_adds (also 2x) down to 28 taps + one final 1x reduce_sum — much
cheaper than a single 1x tensor_reduce over 441. The tree for one of the 3
channels runs on the otherwise-idle gpsimd engine. Softmax normalization is
folded to after the KPN sum (y = sum(patch * exp) / sum(exp)).
"""

import sys

sys.path.insert(0, "/opt/trn_rl_repo")

import numpy as np

import concourse.bass as bass
import concourse.tile as tile
from concourse import mybir
from concourse.bass_utils import run_bass_kernel_spmd

KSIZE = 21
NF = 64
NB = 8
IN_NC = 3
B, H, W = 4, 512, 512
h = w = 128
NCH = KSIZE * KSIZE + 1  # 442
KKP = 448                # padded tap count (zero-padded 441 -> 448)

N_CORES = 8
J = 44    # feature-buffer slots (2 image rows per slot)
X = 130   # 128 cols + 2 zero pad cols

NMID = 2 * NB

_cache = {}


def _enable_ldw_opt():
    import concourse.bass_utils as _bu
    if getattr(_bu, "_ldw_opt_patched", False):
        return
    _orig = _bu.run_command

    def _patched(cmd, **kw):
        if isinstance(cmd, list):
            cmd = ["--enable-ldw-opt=true" if c == "--enable-ldw-opt=false"
                   else c for c in cmd]
        return _orig(cmd, **kw)

    _bu.run_command = _patched
    _bu._ldw_opt_patched = True


def _legalize_waits(nc):
    """This walrus build rejects >1 sync wait per instruction; move extra
    waits onto same-engine NOPs inserted immediately before (engines are
    in-order, so semantics are preserved)."""
    for fn in nc.m.functions:
        for blk in fn.blocks:
            out, changed = [], False
            for inst in blk.instructions:
                si = inst.sync_info
                if si is not None and len(si.on_wait) > 1:
                    waits = list(si.on_wait)
                    for wt in waits[:-1]:
                        nop = mybir.InstNoOp(
                            name=nc.get_next_instruction_name(),
                            ins=[], outs=[], engine=inst.engine)
                        nop.sync_info = mybir.SyncInfo(on_wait=[wt], on_update=[])
                        out.append(nop)
                        changed = True
                    inst.sync_info = mybir.SyncInfo(
                        on_wait=[waits[-1]], on_update=list(si.on_update))
                out.append(inst)
            if changed:
                blk.instructions = out


def _build_nc(bin_zero, bias1_zero, bias2_zero, bout_zero):
    f32 = mybir.dt.float32
    f32r = mybir.dt.float32r
    bf16 = mybir.dt.bfloat16
    nc = bass.Bass()

    zg_e = nc.dram_tensor("zg_e", [IN_NC, 41, 128], f32r, kind="ExternalInput")
    zg_o = nc.dram_tensor("zg_o", [IN_NC, 41, 128], f32r, kind="ExternalInput")
    wl1_in = nc.dram_tensor("wl1_in", [128, 3, 128], f32r, kind="ExternalInput")
    wl2_in = nc.dram_tensor("wl2_in", [128, 3, 128], f32r, kind="ExternalInput")
    wl1_mid = nc.dram_tensor("wl1_mid", [NMID, 128, 3, 128], f32r,
                             kind="ExternalInput")
    wl2_mid = nc.dram_tensor("wl2_mid", [NMID, 128, 3, 128], f32r,
                             kind="ExternalInput")
    wout_lo = nc.dram_tensor("wout_lo", [128, NCH], f32r, kind="ExternalInput")
    wout_hi = nc.dram_tensor("wout_hi", [128, NCH], f32r, kind="ExternalInput")
    biases = nc.dram_tensor("biases", [NMID + 1, 128, 1], f32,
                            kind="ExternalInput")
    bout_r = nc.dram_tensor("bout_r", [1, NCH], f32r, kind="ExternalInput")
    ones_r = nc.dram_tensor("ones_r", [1, 128], f32r, kind="ExternalInput")
    # per-(channel,block) expanded KPN patch windows: xw[c, blk, x0, y, tap]
    # -> per partition (x0) one contiguous 8*448*2 = 7168B descriptor.
    xw = nc.dram_tensor("xw", [IN_NC, 8, 128, 8, KKP], bf16,
                        kind="ExternalInput")

    ydev = nc.dram_tensor("ydev", [128, IN_NC, 64], f32, kind="ExternalOutput")
    nsdev = nc.dram_tensor("nsdev", [128, 64], f32, kind="ExternalOutput")

    with tile.TileContext(nc) as tc:
        wpool = tc.alloc_tile_pool(name="w", bufs=1)
        gpool = tc.alloc_tile_pool(name="g", bufs=1)
        wmpool = tc.alloc_tile_pool(name="wmid", bufs=3)
        tpool = tc.alloc_tile_pool(name="rtmp", bufs=2)
        ppool = tc.alloc_tile_pool(name="patch", bufs=3)
        epool = tc.alloc_tile_pool(name="exp", bufs=2)
        trpool = tc.alloc_tile_pool(name="tree", bufs=1)   # DVE-chained tags
        trpool2 = tc.alloc_tile_pool(name="tree2", bufs=2)  # gpsimd tags
        spool = tc.alloc_tile_pool(name="small", bufs=4)
        psum = tc.alloc_tile_pool(name="ps", bufs=6, space="PSUM")
        psum_o = tc.alloc_tile_pool(name="pso", bufs=2, space="PSUM")

        l1_in = wpool.tile([128, 3, 128], f32r)
        l2_in = wpool.tile([128, 3, 128], f32r)
        wo_lo = wpool.tile([128, NCH], f32r)
        wo_hi = wpool.tile([128, NCH], f32r)
        bias_t = wpool.tile([128, NMID + 1], f32)
        bo_t = wpool.tile([1, NCH], f32r)
        ones_t = wpool.tile([1, 128], f32r)
        nc.sync.dma_start(out=l1_in, in_=wl1_in[:])
        nc.sync.dma_start(out=l2_in, in_=wl2_in[:])
        nc.sync.dma_start(out=wo_lo, in_=wout_lo[:])
        nc.sync.dma_start(out=wo_hi, in_=wout_hi[:])
        nc.sync.dma_start(out=bias_t,
                          in_=biases[:].rearrange("l p one -> p (l one)"))
        nc.sync.dma_start(out=bo_t, in_=bout_r[:])
        nc.sync.dma_start(out=ones_t, in_=ones_r[:])

        g_z = gpool.tile([128, J, X], f32r)
        feat = gpool.tile([128, J, X], f32r)
        t1 = gpool.tile([128, J, X], f32r)
        # g_z must be fully zero (unused input-channel partitions are read by
        # the K=128 contraction of the input conv; NaN * 0-weight = NaN).
        nc.vector.memset(g_z[:].bitcast(mybir.dt.float32), 0.0)
        # feat / t1 only need the pad columns (x=0, x=129, read as the dx
        # halo with nonzero weights) and t1's never-written top slots (read
        # one slot past the shrinking write front) cleared; everything else
        # is overwritten before being read.
        for buf in (feat, t1):
            nc.gpsimd.memset(buf[:, :, 0:1].bitcast(mybir.dt.float32), 0.0)
            nc.gpsimd.memset(buf[:, :, 129:130].bitcast(mybir.dt.float32), 0.0)
            # slots 0:2 hold the virtual rows above the image for the upper
            # (odd-parity) half — read with nonzero weights, so must be zero
            nc.gpsimd.memset(buf[:, 0:2, :].bitcast(mybir.dt.float32), 0.0)
        nc.gpsimd.memset(t1[:, 41:44, :].bitcast(mybir.dt.float32), 0.0)
        nc.gpsimd.memset(feat[:, 42:44, :].bitcast(mybir.dt.float32), 0.0)

        # z rows (shard-local, 0..80): even row r -> partitions 0:3 slot
        # r//2+1; odd row r -> partitions 64:67 slot (r+1)//2+1.
        # Host packs zg_e = even rows 0..80 (slots 1..41), zg_o = odd rows
        # 1..79 at slots 2..41 (zg_o[0] stays zero -> slot 1 zero = row -1).
        nc.sync.dma_start(out=g_z[0:IN_NC, 1:42, 1:129], in_=zg_e[:])
        nc.sync.dma_start(out=g_z[64:64 + IN_NC, 1:42, 1:129], in_=zg_o[:])

        relu = mybir.ActivationFunctionType.Relu
        ident = mybir.ActivationFunctionType.Identity

        def conv(src, dst, l1, l2, bias_col, func, k_halo, residual,
                 post_group=None, zero_bias=False):
            # output region: shard-local rows 0 .. 63 + k_halo -> slots 1..hi
            hi = (64 + k_halo) // 2 + 1      # top slot of even output rows
            blocks = [(s, min(4, hi - s + 1)) for s in range(1, hi + 1, 4)]
            # weight-major inside groups of 5 blocks: consecutive matmuls
            # share the stationary operand so walrus ldw-opt dedups the
            # (serialized, non-overlapping) LDWEIGHTS streams.
            mx = mybir.AluOpType.max
            for gi, g0 in enumerate(range(0, len(blocks), 5)):
                grp = blocks[g0:g0 + 5]
                tiles = [psum.tile([128, 4, 128], f32, tag="convps",
                                   name=f"cps_{g0}_{i}")
                         for i in range(len(grp))]
                for wi in range(6):
                    dx, phase = wi % 3, wi // 3
                    wt = (l1 if phase == 0 else l2)[:, dx]
                    for (s0, mc), P in zip(grp, tiles):
                        o = s0 + phase
                        nc.tensor.matmul(
                            P[:, 0:mc], wt,
                            src[0:128, o:o + mc, dx:dx + 128],
                            start=(wi == 0), stop=(wi == 5))
                # Evacuation: one reader engine per PSUM tile (dual readers
                # add waits to the tensor queue and stall the matmul stream).
                # Non-residual convs with zero bias alternate groups between
                # the scalar engine (ACT) and DVE (tensor_scalar max(x,0)
                # covers both relu and identity since bias is zero).
                dve_grp = residual is None and zero_bias and gi % 2 == 1
                for (s0, mc), P in zip(grp, tiles):
                    if residual is None and dve_grp:
                        for pr, dr in (((0, 64), dst[0:64, s0:s0 + mc, 1:129]),
                                       ((64, 128),
                                        dst[64:128, s0 + 1:s0 + 1 + mc, 1:129])):
                            if func is relu:
                                nc.vector.tensor_scalar(
                                    out=dr, in0=P[pr[0]:pr[1], 0:mc],
                                    scalar1=0.0, scalar2=None, op0=mx)
                            else:
                                nc.vector.tensor_copy(
                                    dr, P[pr[0]:pr[1], 0:mc])
                    elif residual is None:
                        nc.scalar.activation(
                            out=dst[0:64, s0:s0 + mc, 1:129],
                            in_=P[0:64, 0:mc],
                            func=func, bias=bias_col[0:64], scale=1.0)
                        nc.scalar.activation(
                            out=dst[64:128, s0 + 1:s0 + 1 + mc, 1:129],
                            in_=P[64:128, 0:mc],
                            func=func, bias=bias_col[64:128], scale=1.0)
                    else:
                        # single ACT evacuates the tile; residual adds go to
                        # DVE (lower) and gpsimd (upper), out-of-place into
                        # the other trunk buffer.
                        tmp = tpool.tile([128, 4, 128], f32, tag="rtmp")
                        nc.scalar.activation(
                            out=tmp[:, 0:mc], in_=P[:, 0:mc], func=ident,
                            bias=0.0 if bias2_zero else bias_col, scale=1.0)
                        nc.vector.tensor_add(
                            out=dst[0:64, s0:s0 + mc, 1:129],
                            in0=tmp[0:64, 0:mc],
                            in1=residual[0:64, s0:s0 + mc, 1:129])
                        nc.gpsimd.tensor_add(
                            out=dst[64:128, s0 + 1:s0 + 1 + mc, 1:129],
                            in0=tmp[64:128, 0:mc],
                            in1=residual[64:128, s0 + 1:s0 + 1 + mc, 1:129])
                if post_group is not None:
                    s_last, mc_last = grp[-1]
                    post_group(s_last + mc_last - 1)

        yacc = spool.tile([128, IN_NC, 64], f32, tag="yacc")
        nsacc = spool.tile([128, 64], f32, tag="nsacc")

        KK = KSIZE * KSIZE
        dma_engines = [nc.sync, nc.scalar, nc.gpsimd]

        def kpn_block(bi, final_feat):
            y0 = bi * 8
            ex2 = epool.tile([128, 8, NCH], bf16, tag="ex")
            ssum2 = spool.tile([128, 8], f32, tag="ssum")
            patches = []
            for c in range(IN_NC):
                patch2 = ppool.tile([128, 8, KKP], bf16, tag="patch")
                dma_engines[c].dma_start(out=patch2, in_=xw[c, bi])
                patches.append(patch2)
            for r in range(8):
                yl = y0 + r
                if yl % 2 == 0:
                    slot, wsel = yl // 2 + 1, wo_lo
                else:
                    slot, wsel = (yl + 1) // 2 + 1, wo_hi
                Po = psum_o.tile([128, NCH], f32, tag="pout")
                nc.tensor.matmul(Po, final_feat[:, slot, 1:129], wsel,
                                 start=True, stop=bout_zero)
                if not bout_zero:
                    nc.tensor.matmul(Po, ones_t, bo_t, start=False, stop=True)
                nc.scalar.activation(out=ex2[:, r], in_=Po,
                                     func=mybir.ActivationFunctionType.Exp,
                                     scale=1.0, accum_out=ssum2[:, r:r + 1])
            rcp2 = spool.tile([128, 8], f32, tag="rcp")
            nc.vector.reciprocal(out=rcp2, in_=ssum2)
            ex2v = ex2[:, :, 0:KK]
            for c in range(IN_NC):
                eng = nc.gpsimd if c == 2 else nc.vector
                pool_c = trpool2 if c == 2 else trpool
                prod2 = pool_c.tile([128, 8, KKP], bf16, tag=f"prod{c}")
                # taps 441..447 of patch are zero -> prod padding is zero
                nc.vector.tensor_mul(out=prod2[:, :, 0:KK], in0=ex2v,
                                     in1=patches[c][:, :, 0:KK])
                nc.vector.tensor_mul(out=prod2[:, :, KK:KKP],
                                     in0=patches[c][:, :, KK:KKP],
                                     in1=patches[c][:, :, KK:KKP])
                # pairwise tree at DVE 2x (vs 1x tensor_reduce); scratch is a
                # two-tile pyramid: s1 <- halves of prod, then reuse dead
                # ranges of prod/s1 for the later (smaller) levels.
                s1 = pool_c.tile([128, 8, 224], bf16, tag=f"s1_{c}")
                eng.tensor_add(out=s1, in0=prod2[:, :, 0:224],
                               in1=prod2[:, :, 224:448])
                eng.tensor_add(out=prod2[:, :, 0:112], in0=s1[:, :, 0:112],
                               in1=s1[:, :, 112:224])
                eng.tensor_add(out=s1[:, :, 0:56], in0=prod2[:, :, 0:56],
                               in1=prod2[:, :, 56:112])
                eng.tensor_add(out=prod2[:, :, 0:28], in0=s1[:, :, 0:28],
                               in1=s1[:, :, 28:56])
                pc2 = spool.tile([128, 8, 1], f32, tag=f"pc{c}")
                nc.vector.reduce_sum(out=pc2, in_=prod2[:, :, 0:28],
                                     axis=mybir.AxisListType.X)
                nc.vector.tensor_mul(out=yacc[:, c, y0:y0 + 8],
                                     in0=pc2[:, :, 0], in1=rcp2)
            nc.vector.tensor_mul(out=nsacc[:, y0:y0 + 8],
                                 in0=ex2[:, :, NCH - 1], in1=rcp2)

        conv(g_z, feat, l1_in, l2_in, bias_t[:, 0:1], ident, 16, None,
             zero_bias=bin_zero)
        trunk = [feat, g_z]   # ping-pong; g_z recycled (pads already zero)
        for rb in range(NB):
            src = trunk[rb % 2]
            dst = trunk[(rb + 1) % 2]
            la, lb = 2 * rb, 2 * rb + 1
            w1a = wmpool.tile([128, 3, 128], f32r, tag="w1")
            w2a = wmpool.tile([128, 3, 128], f32r, tag="w2")
            nc.sync.dma_start(out=w1a, in_=wl1_mid[la])
            nc.sync.dma_start(out=w2a, in_=wl2_mid[la])
            conv(src, t1, w1a, w2a,
                 bias_t[:, 1 + la:2 + la], relu, 15 - 2 * rb, None,
                 zero_bias=bias1_zero)
            w1b = wmpool.tile([128, 3, 128], f32r, tag="w1")
            w2b = wmpool.tile([128, 3, 128], f32r, tag="w2")
            nc.sync.dma_start(out=w1b, in_=wl1_mid[lb])
            nc.sync.dma_start(out=w2b, in_=wl2_mid[lb])
            if rb < NB - 1:
                conv(t1, dst, w1b, w2b,
                     bias_t[:, 1 + lb:2 + lb], ident, 14 - 2 * rb, src)
            else:
                # final conv: interleave KPN blocks as their slots complete
                final_feat = dst
                state = {"next": 0}

                def on_group(s_done, _ff=final_feat, _st=state):
                    # kpn block b needs slots up to 4b+5 (odd rows skew +1)
                    while _st["next"] < 8 and 4 * _st["next"] + 5 <= s_done:
                        kpn_block(_st["next"], _ff)
                        _st["next"] += 1

                conv(t1, dst, w1b, w2b,
                     bias_t[:, 1 + lb:2 + lb], ident, 14 - 2 * rb, src,
                     post_group=on_group)
                while state["next"] < 8:
                    kpn_block(state["next"], final_feat)
                    state["next"] += 1

        nc.sync.dma_start(out=ydev[:], in_=yacc)
        nc.sync.dma_start(out=nsdev[:], in_=nsacc)

        for p in (psum_o, psum, spool, trpool2, trpool, epool, ppool, tpool,
                  wmpool, gpool, wpool):
            p.release()

    _legalize_waits(nc)
    return nc


def _stack_l1l2(Wl):
    # Wl [64o, ic, 3, 3] -> L1, L2 [128, 3, 128]
    ic = Wl.shape[1]
    L1 = np.zeros((128, 3, 128), np.float32)
    L2 = np.zeros((128, 3, 128), np.float32)
    for dx in range(3):
        L1[0:ic, dx, 0:64] = Wl[:, :, 1, dx].T
        L1[64:64 + ic, dx, 0:64] = Wl[:, :, 0, dx].T
        L1[0:ic, dx, 64:128] = Wl[:, :, 0, dx].T
        L2[64:64 + ic, dx, 0:64] = Wl[:, :, 2, dx].T
        L2[0:ic, dx, 64:128] = Wl[:, :, 2, dx].T
        L2[64:64 + ic, dx, 64:128] = Wl[:, :, 1, dx].T
    return L1, L2


def _prep_weights(w_in, w1s, w2s, w_out, flip):
    if flip:
        w_in = w_in[:, :, ::-1, :]
        w1s = w1s[:, :, :, ::-1, :]
        w2s = w2s[:, :, :, ::-1, :]
    l1_in, l2_in = _stack_l1l2(w_in)
    L1m = np.zeros((NMID, 128, 3, 128), np.float32)
    L2m = np.zeros((NMID, 128, 3, 128), np.float32)
    for rb in range(NB):
        L1m[2 * rb], L2m[2 * rb] = _stack_l1l2(w1s[rb])
        L1m[2 * rb + 1], L2m[2 * rb + 1] = _stack_l1l2(w2s[rb])
    wo = w_out[:, :, 0, 0]  # [442, 64]
    wlo = np.zeros((128, NCH), np.float32)
    whi = np.zeros((128, NCH), np.float32)
    wlo[0:64] = wo.T
    whi[64:128] = wo.T
    return l1_in, l2_in, L1m, L2m, wlo, whi


def kernel(x, z, eps, w_in, b_in, w1s, b1s, w2s, b2s, w_out, b_out):
    x = np.ascontiguousarray(np.asarray(x, np.float32))
    z = np.asarray(z, np.float32)
    eps = np.asarray(eps, np.float32)
    w_in = np.asarray(w_in, np.float32)
    b_in = np.asarray(b_in, np.float32)
    w1s = np.asarray(w1s, np.float32)
    b1s = np.asarray(b1s, np.float32)
    w2s = np.asarray(w2s, np.float32)
    b2s = np.asarray(b2s, np.float32)
    w_out = np.asarray(w_out, np.float32)
    b_out = np.asarray(b_out, np.float32)

    bin_zero = bool(np.all(b_in == 0))
    bias1_zero = bool(np.all(b1s == 0))
    bias2_zero = bool(np.all(b2s == 0))
    bout_zero = bool(np.all(b_out == 0))
    _enable_ldw_opt()
    key = (bin_zero, bias1_zero, bias2_zero, bout_zero)
    if key not in _cache:
        _cache[key] = _build_nc(bin_zero, bias1_zero, bias2_zero, bout_zero)
    nc = _cache[key]

    weights = {}
    for flip in (False, True):
        l1_in, l2_in, L1m, L2m, wlo, whi = _prep_weights(
            w_in, w1s, w2s, w_out, flip)
        weights[flip] = (l1_in, l2_in, L1m, L2m, wlo, whi)

    biases = np.zeros((NMID + 1, 128, 1), np.float32)
    biases[0, 0:64, 0] = b_in
    biases[0, 64:128, 0] = b_in
    for rb in range(NB):
        biases[1 + 2 * rb, 0:64, 0] = b1s[rb]
        biases[1 + 2 * rb, 64:128, 0] = b1s[rb]
        biases[2 + 2 * rb, 0:64, 0] = b2s[rb]
        biases[2 + 2 * rb, 64:128, 0] = b2s[rb]
    bout_row = np.ascontiguousarray(b_out.reshape(1, NCH))
    ones_row = np.ones((1, 128), np.float32)

    # padded x (vertical dim only logical; we slice rows directly)
    in_maps = []
    for core in range(N_CORES):
        b, half = core // 2, core % 2
        flip = half == 1
        # shard-local z rows 0..80: top zl[r] = z[b, r]; bottom z flipped
        zl = z[b] if not flip else z[b, :, ::-1]
        zg_e = np.zeros((IN_NC, 41, 128), np.float32)
        zg_o = np.zeros((IN_NC, 41, 128), np.float32)
        zg_e[:, 0:41] = zl[:, 0:81:2]          # rows 0,2,..,80 -> slots 1..41
        zg_o[:, 1:41] = zl[:, 1:80:2]          # rows 1,3,..,79 -> slots 2..41
        # KPN patch windows, fully expanded per output row:
        # xw[c, blk, x0, y, t] = xp[c, 4*y0(8*blk+y) + t//21? ...] laid out as
        # [3, 8, 128, 8, 448]: tap index t < 441 maps to (t//21, t%21) of the
        # 21x21 window; taps 441..447 are zero padding.
        import ml_dtypes
        xp = np.zeros((IN_NC, H + 2 * 10, W + 2 * 10), dtype=ml_dtypes.bfloat16)
        xp[:, 10:10 + H, 10:10 + W] = x[b]
        y0s = np.arange(64) if not flip else (127 - np.arange(64))
        ridx = (4 * y0s)[:, None] + np.arange(KSIZE)[None, :]   # [64, 21]
        cols = 4 * np.arange(128)[:, None] + np.arange(KSIZE)[None, :]
        sub = xp[:, ridx]                 # [3, 64, 21, 532]
        sub = sub[:, :, :, cols]          # [3, 64, 21, 128, 21]
        xw_arr = np.zeros((IN_NC, 64, 128, KKP), dtype=ml_dtypes.bfloat16)
        xw_arr[:, :, :, 0:KSIZE * KSIZE] = np.transpose(
            sub, (0, 1, 3, 2, 4)).reshape(IN_NC, 64, 128, KSIZE * KSIZE)
        xw_arr = np.ascontiguousarray(
            xw_arr.reshape(IN_NC, 8, 8, 128, KKP).transpose(0, 1, 3, 2, 4))
        l1_in, l2_in, L1m, L2m, wlo, whi = weights[flip]
        in_maps.append({
            "zg_e": zg_e, "zg_o": zg_o,
            "wl1_in": l1_in, "wl2_in": l2_in,
            "wl1_mid": L1m, "wl2_mid": L2m,
            "wout_lo": wlo, "wout_hi": whi,
            "biases": biases, "bout_r": bout_row, "ones_r": ones_row,
            "xw": xw_arr,
        })

    trace = bool(globals().get("TRACE", False))
    res = run_bass_kernel_spmd(nc, in_maps, core_ids=list(range(N_CORES)),
                               trace=trace)
    globals()["_last_result"] = res

    out = np.zeros((B, IN_NC, h, w), np.float32)
    for bb in range(B):
        ns_sum = (float(res.results[2 * bb]["nsdev"].sum())
                  + float(res.results[2 * bb + 1]["nsdev"].sum()))
        mean_ns = ns_sum / (h * w)
        for half in range(2):
            ydev = res.results[2 * bb + half]["ydev"]  # [128, 3, 64]
            yt = np.transpose(ydev, (1, 2, 0))         # [3, 64, 128]
            if half == 0:
                out[bb, :, 0:64, :] = yt
            else:
                out[bb, :, 64:128, :] = yt[:, ::-1, :]
        out[bb] += mean_ns * eps[bb]
    return out
